# revision 6
# baseline (speedup 1.0000x reference)
"""CycleFC (1-bit weights/activations) Trainium2 kernel.

Computes, for x (B=32, C=384, H=56, W=56), weight (C, C), bias (C,):
    xb = sign(x); wb = sign(weight)
    shifted[b,c,h,w] = xb[b,c,h,w+dx_c]  (0 outside [0,W)), dx_c = (c+3)%7-3
    out = einsum('bchw,oc->bohw', shifted, wb) + bias

Strategy (8 NeuronCores, SPMD):
  - Data-parallel over batch: 4 batches per core; weight/bias replicated.
  - The host stores each 56-wide row padded to 59 with zeros.  The
    per-channel horizontal shift then folds into the input DMA for free:
    for a fixed shift dx, the shifted plane is just the flat padded plane
    read at offset +dx -- positions that fall outside [0, W) pick up the
    row padding, which is exactly the required zero padding.  The matmul
    reads [8 rows x 56 cols] strided views, never touching pad columns.
  - Channels are processed in a permuted order (grouped by c mod 7 ==
    constant shift) so each shift group is a partition-contiguous,
    channel-stride-7 affine DMA segment.  The weight matrix is permuted
    identically on the host (pure layout transform, no arithmetic), which
    leaves the GEMM result unchanged.
  - sign() runs on the Scalar engine (fp32 -> bf16; +-1 is exact in bf16
    and the 384-term accumulation is exact in fp32 PSUM, so the result is
    bit-identical to an fp32 computation).
  - GEMM: out[o, p] = sum_c wbT[c, o] * xb[c, p] on the Tensor engine,
    K = 384 contracted in 3 chunks of 128, N tiles of 448 (8 H-rows).
  - Bias add fused into the PSUM -> SBUF copy on the Vector engine.
"""

import numpy as np

import concourse.bass as bass
import concourse.tile as tile
from concourse import bacc, mybir
from concourse.bass_utils import run_bass_kernel_spmd

# Problem constants (hardcoded per spec)
B, C, H, W = 32, 384, 56, 56
PLANE = H * W              # 3136 (unpadded output plane)
NCORES = 8
BL = B // NCORES           # 4 batches per core
KS = 7                     # cyclic shift period (kernel_size 7)
NK = C // 128              # 3 contraction chunks
NM = C // 128              # 3 output-channel chunks
ROWS_PER_TILE = 8
NTILE = ROWS_PER_TILE * W  # 448 pixels per PSUM tile
NN = H // ROWS_PER_TILE    # 7 pixel tiles per (b, m)
WPAD = 59                  # row pitch: 56 data + 3 zero cols (>= max |dx|)
PLANE_P = H * WPAD         # 3304 (padded input plane)
BACK_PAD = 7 * PLANE_P     # slack so segment APs can over-claim past the end
NX_ELEMS = BL * C * PLANE_P + BACK_PAD
NOUT_ELEMS = BL * C * PLANE

# Shift-group segments of the permuted channel order.  perm = channels
# grouped by r = c mod 7 (r ascending, then c ascending within the group).
# Each segment is a partition-contiguous run inside one 128-channel chunk:
# (chunk, part_start, nseg, c_first, dx) with original channels
# c_first + 7*i for i in [0, nseg).
SEGMENTS = [
    (0, 0, 55, 0, 0),
    (0, 55, 55, 1, 1),
    (0, 110, 18, 2, 2),
    (1, 0, 37, 128, 2),    # r=2 continued: 2 + 7*18
    (1, 37, 55, 3, 3),
    (1, 92, 36, 4, -3),
    (2, 0, 19, 256, -3),   # r=4 continued: 4 + 7*36
    (2, 19, 55, 5, -2),
    (2, 74, 54, 6, -1),
]

PERM = np.concatenate([np.arange(r, C, KS) for r in range(KS)])

_COMPILED = None


def _build_program():
    """Trace + compile the single-core Bass program (same on all 8 cores)."""
    nc = bacc.Bacc(
        "TRN2",
        target_bir_lowering=False,
        debug=False,
        num_devices=NCORES,
    )
    x_d = nc.dram_tensor("x", [NX_ELEMS], mybir.dt.float32, kind="ExternalInput")
    w_d = nc.dram_tensor("wt", [C, C], mybir.dt.float32, kind="ExternalInput")
    b_d = nc.dram_tensor("bias", [C], mybir.dt.float32, kind="ExternalInput")
    o_d = nc.dram_tensor("out", [NOUT_ELEMS], mybir.dt.float32, kind="ExternalOutput")

    x_ap = x_d.ap()
    o_ap = o_d.ap()

    segs_by_chunk = [[s[1:] for s in SEGMENTS if s[0] == k] for k in range(NK)]

    with tile.TileContext(nc) as tc:
        with (
            tc.tile_pool(name="const", bufs=1) as cpool,
            tc.tile_pool(name="xraw", bufs=4) as xraw_pool,
            tc.tile_pool(name="xb", bufs=6) as xb_pool,
            tc.tile_pool(name="psum", bufs=4, space="PSUM") as psum_pool,
            tc.tile_pool(name="outs", bufs=4) as out_pool,
        ):
            # Binarized, pre-transposed, channel-permuted weights: wbT[c, o].
            w_bf = []
            for k in range(NK):
                wraw = cpool.tile([128, C], mybir.dt.float32, tag=f"wraw{k}")
                nc.sync.dma_start(wraw[:], w_d.ap()[128 * k : 128 * (k + 1), :])
                wb = cpool.tile([128, C], mybir.dt.bfloat16, tag=f"wb{k}")
                nc.scalar.sign(wb[:], wraw[:])
                w_bf.append(wb)

            bias_t = []
            for m in range(NM):
                bt = cpool.tile([128, 1], mybir.dt.float32, tag=f"bias{m}")
                nc.sync.dma_start(bt[:], b_d.ap()[128 * m : 128 * (m + 1)].unsqueeze(1))
                bias_t.append(bt)

            for b in range(BL):
                xbs = []
                for k in range(NK):
                    xr = xraw_pool.tile([128, PLANE_P], mybir.dt.float32, tag="xr")
                    for (part_start, nseg, c_first, dx) in segs_by_chunk[k]:
                        base = (b * C + c_first) * PLANE_P + dx
                        src = (
                            x_ap[base : base + nseg * KS * PLANE_P]
                            .rearrange("(p q) -> p q", q=KS * PLANE_P)[:, :PLANE_P]
                        )
                        nc.sync.dma_start(xr[part_start : part_start + nseg, :], src)

                    xb = xb_pool.tile([128, PLANE_P], mybir.dt.bfloat16, tag="xb")
                    nc.scalar.sign(xb[:], xr[:])
                    xbs.append(xb)

                for m in range(NM):
                    for n in range(NN):
                        ps = psum_pool.tile([128, NTILE], mybir.dt.float32, tag="ps")
                        for k in range(NK):
                            rhs = xbs[k][:].rearrange("p (h w) -> p h w", w=WPAD)[
                                :, ROWS_PER_TILE * n : ROWS_PER_TILE * (n + 1), :W
                            ]
                            nc.tensor.matmul(
                                ps[:],
                                w_bf[k][:, 128 * m : 128 * (m + 1)],
                                rhs,
                                start=(k == 0),
                                stop=(k == NK - 1),
                            )
                        ot = out_pool.tile([128, NTILE], mybir.dt.float32, tag="ot")
                        nc.vector.tensor_scalar_add(ot[:], ps[:], bias_t[m][:])
                        obase = (b * C + 128 * m) * PLANE
                        dst = (
                            o_ap[obase : obase + 128 * PLANE]
                            .rearrange("(p q) -> p q", q=PLANE)
                            [:, NTILE * n : NTILE * (n + 1)]
                        )
                        nc.sync.dma_start(dst, ot[:])

    nc.compile()
    return nc


def _get_program():
    global _COMPILED
    if _COMPILED is None:
        _COMPILED = _build_program()
    return _COMPILED


# Set by test harness to request an NTFF-profiled run; results stashed here.
TRACE = False
LAST_EXEC_TIME_NS = None


def kernel(x, weight, bias):
    global LAST_EXEC_TIME_NS
    x = np.ascontiguousarray(np.asarray(x, dtype=np.float32))
    weight = np.asarray(weight, dtype=np.float32)
    bias = np.ascontiguousarray(np.asarray(bias, dtype=np.float32))

    # Pure layout transform (no arithmetic): transpose + channel-permute the
    # weight so device partition p of contraction chunk k holds original
    # channel PERM[128k + p], matching the activation segment layout.
    wtp = np.ascontiguousarray(weight[:, PERM].T)

    nc = _get_program()

    in_maps = []
    for i in range(NCORES):
        xi = np.zeros(NX_ELEMS, dtype=np.float32)
        view = xi[: BL * C * PLANE_P].reshape(BL, C, H, WPAD)
        view[..., :W] = x[i * BL : (i + 1) * BL]
        in_maps.append({"x": xi, "wt": wtp, "bias": bias})

    res = run_bass_kernel_spmd(
        nc, in_maps, list(range(NCORES)), trace=TRACE
    )
    LAST_EXEC_TIME_NS = res.exec_time_ns

    out = np.empty((B, C, H, W), dtype=np.float32)
    for i in range(NCORES):
        out[i * BL : (i + 1) * BL] = res.results[i]["out"].reshape(BL, C, H, W)
    return out


# revision 8
# speedup vs baseline: 1.0765x; 1.0765x over previous
"""CycleFC (1-bit weights/activations) Trainium2 kernel.

Computes, for x (B=32, C=384, H=56, W=56), weight (C, C), bias (C,):
    xb = sign(x); wb = sign(weight)
    shifted[b,c,h,w] = xb[b,c,h,w+dx_c]  (0 outside [0,W)), dx_c = (c+3)%7-3
    out = einsum('bchw,oc->bohw', shifted, wb) + bias

Strategy (8 NeuronCores, SPMD):
  - Data-parallel over batch: 4 batches per core; weight/bias replicated.
  - The host stores each 56-wide row padded to 59 with zeros.  The
    per-channel horizontal shift then folds into the input DMA for free:
    for a fixed shift dx, the shifted plane is just the flat padded plane
    read at offset +dx -- positions that fall outside [0, W) pick up the
    row padding, which is exactly the required zero padding.  The matmul
    reads [8 rows x 56 cols] strided views, never touching pad columns.
  - Channels are processed in a permuted order (grouped by c mod 7 ==
    constant shift) so each shift group is a partition-contiguous,
    channel-stride-7 affine DMA segment.  The weight matrix is permuted
    identically on the host (pure layout transform, no arithmetic), which
    leaves the GEMM result unchanged.
  - sign() runs on the Scalar engine (fp32 -> bf16; +-1 is exact in bf16
    and the 384-term accumulation is exact in fp32 PSUM, so the result is
    bit-identical to an fp32 computation).
  - GEMM: out[o, p] = sum_c wbT[c, o] * xb[c, p] on the Tensor engine,
    K = 384 contracted in 3 chunks of 128, N tiles of 448 (8 H-rows).
  - Bias add fused into the PSUM -> SBUF copy on the Vector engine.
"""

import numpy as np

import concourse.bass as bass
import concourse.tile as tile
from concourse import bacc, mybir
from concourse.bass_utils import run_bass_kernel_spmd

# Problem constants (hardcoded per spec)
B, C, H, W = 32, 384, 56, 56
PLANE = H * W              # 3136 (unpadded output plane)
NCORES = 8
BL = B // NCORES           # 4 batches per core
KS = 7                     # cyclic shift period (kernel_size 7)
NK = C // 128              # 3 contraction chunks
NM = C // 128              # 3 output-channel chunks
ROWS_PER_TILE = 8
NTILE = ROWS_PER_TILE * W  # 448 pixels per PSUM tile
NN = H // ROWS_PER_TILE    # 7 pixel tiles per (b, m)
WPAD = 59                  # row pitch: 56 data + 3 zero cols (>= max |dx|)
PLANE_P = H * WPAD         # 3304 (padded input plane)
BACK_PAD = 7 * PLANE_P     # slack so segment APs can over-claim past the end
NX_ELEMS = BL * C * PLANE_P + BACK_PAD
NOUT_ELEMS = BL * C * PLANE

# Shift-group segments of the permuted channel order.  perm = channels
# grouped by r = c mod 7 (r ascending, then c ascending within the group).
# Each segment is a partition-contiguous run inside one 128-channel chunk:
# (chunk, part_start, nseg, c_first, dx) with original channels
# c_first + 7*i for i in [0, nseg).
SEGMENTS = [
    (0, 0, 55, 0, 0),
    (0, 55, 55, 1, 1),
    (0, 110, 18, 2, 2),
    (1, 0, 37, 128, 2),    # r=2 continued: 2 + 7*18
    (1, 37, 55, 3, 3),
    (1, 92, 36, 4, -3),
    (2, 0, 19, 256, -3),   # r=4 continued: 4 + 7*36
    (2, 19, 55, 5, -2),
    (2, 74, 54, 6, -1),
]

PERM = np.concatenate([np.arange(r, C, KS) for r in range(KS)])

_COMPILED = None


def _build_program():
    """Trace + compile the single-core Bass program (same on all 8 cores)."""
    nc = bacc.Bacc(
        "TRN2",
        target_bir_lowering=False,
        debug=False,
        num_devices=NCORES,
    )
    x_d = nc.dram_tensor("x", [NX_ELEMS], mybir.dt.float32, kind="ExternalInput")
    w_d = nc.dram_tensor("wt", [C, C], mybir.dt.float32, kind="ExternalInput")
    b_d = nc.dram_tensor("bias", [C], mybir.dt.float32, kind="ExternalInput")
    o_d = nc.dram_tensor("out", [NOUT_ELEMS], mybir.dt.float32, kind="ExternalOutput")

    x_ap = x_d.ap()
    o_ap = o_d.ap()

    segs_by_chunk = [[s[1:] for s in SEGMENTS if s[0] == k] for k in range(NK)]

    with tile.TileContext(nc) as tc:
        with (
            tc.tile_pool(name="const", bufs=1) as cpool,
            tc.tile_pool(name="xraw", bufs=6) as xraw_pool,
            tc.tile_pool(name="xb", bufs=6) as xb_pool,
            tc.tile_pool(name="psum", bufs=8, space="PSUM") as psum_pool,
            tc.tile_pool(name="outs", bufs=8) as out_pool,
        ):
            # Binarized, pre-transposed, channel-permuted weights: wbT[c, o].
            w_bf = []
            for k in range(NK):
                wraw = cpool.tile([128, C], mybir.dt.float32, tag=f"wraw{k}")
                nc.sync.dma_start(wraw[:], w_d.ap()[128 * k : 128 * (k + 1), :])
                wb = cpool.tile([128, C], mybir.dt.bfloat16, tag=f"wb{k}")
                nc.scalar.sign(wb[:], wraw[:])
                w_bf.append(wb)

            bias_t = []
            for m in range(NM):
                bt = cpool.tile([128, 1], mybir.dt.float32, tag=f"bias{m}")
                nc.sync.dma_start(bt[:], b_d.ap()[128 * m : 128 * (m + 1)].unsqueeze(1))
                bias_t.append(bt)

            for b in range(BL):
                xbs = []
                for k in range(NK):
                    xr = xraw_pool.tile([128, PLANE_P], mybir.dt.float32, tag="xr")
                    for (part_start, nseg, c_first, dx) in segs_by_chunk[k]:
                        base = (b * C + c_first) * PLANE_P + dx
                        src = (
                            x_ap[base : base + nseg * KS * PLANE_P]
                            .rearrange("(p q) -> p q", q=KS * PLANE_P)[:, :PLANE_P]
                        )
                        nc.sync.dma_start(xr[part_start : part_start + nseg, :], src)

                    xb = xb_pool.tile([128, PLANE_P], mybir.dt.bfloat16, tag="xb")
                    nc.scalar.sign(xb[:], xr[:])
                    xbs.append(xb)

                for m in range(NM):
                    for n in range(NN):
                        ps = psum_pool.tile([128, NTILE], mybir.dt.float32, tag="ps")
                        for k in range(NK):
                            rhs = xbs[k][:].rearrange("p (h w) -> p h w", w=WPAD)[
                                :, ROWS_PER_TILE * n : ROWS_PER_TILE * (n + 1), :W
                            ]
                            nc.tensor.matmul(
                                ps[:],
                                w_bf[k][:, 128 * m : 128 * (m + 1)],
                                rhs,
                                start=(k == 0),
                                stop=(k == NK - 1),
                            )
                        ot = out_pool.tile([128, NTILE], mybir.dt.float32, tag="ot")
                        nc.vector.tensor_scalar_add(ot[:], ps[:], bias_t[m][:])
                        obase = (b * C + 128 * m) * PLANE
                        dst = (
                            o_ap[obase : obase + 128 * PLANE]
                            .rearrange("(p q) -> p q", q=PLANE)
                            [:, NTILE * n : NTILE * (n + 1)]
                        )
                        # SWDGE: keeps store descriptor-gen off the Sync
                        # engine's HWDGE ring, which the loads use.
                        nc.gpsimd.dma_start(dst, ot[:])

    nc.compile()
    return nc


def _get_program():
    global _COMPILED
    if _COMPILED is None:
        _COMPILED = _build_program()
    return _COMPILED


# Set by test harness to request an NTFF-profiled run; results stashed here.
TRACE = False
LAST_EXEC_TIME_NS = None


def kernel(x, weight, bias):
    global LAST_EXEC_TIME_NS
    x = np.ascontiguousarray(np.asarray(x, dtype=np.float32))
    weight = np.asarray(weight, dtype=np.float32)
    bias = np.ascontiguousarray(np.asarray(bias, dtype=np.float32))

    # Pure layout transform (no arithmetic): transpose + channel-permute the
    # weight so device partition p of contraction chunk k holds original
    # channel PERM[128k + p], matching the activation segment layout.
    wtp = np.ascontiguousarray(weight[:, PERM].T)

    nc = _get_program()

    in_maps = []
    for i in range(NCORES):
        xi = np.zeros(NX_ELEMS, dtype=np.float32)
        view = xi[: BL * C * PLANE_P].reshape(BL, C, H, WPAD)
        view[..., :W] = x[i * BL : (i + 1) * BL]
        in_maps.append({"x": xi, "wt": wtp, "bias": bias})

    res = run_bass_kernel_spmd(
        nc, in_maps, list(range(NCORES)), trace=TRACE
    )
    LAST_EXEC_TIME_NS = res.exec_time_ns

    out = np.empty((B, C, H, W), dtype=np.float32)
    for i in range(NCORES):
        out[i * BL : (i + 1) * BL] = res.results[i]["out"].reshape(BL, C, H, W)
    return out


# revision 10
# speedup vs baseline: 1.6143x; 1.4997x over previous
"""CycleFC (1-bit weights/activations) Trainium2 kernel.

Computes, for x (B=32, C=384, H=56, W=56), weight (C, C), bias (C,):
    xb = sign(x); wb = sign(weight)
    shifted[b,c,h,w] = xb[b,c,h,w+dx_c]  (0 outside [0,W)), dx_c = (c+3)%7-3
    out = einsum('bchw,oc->bohw', shifted, wb) + bias

Strategy (8 NeuronCores, SPMD):
  - Data-parallel over batch: 4 batches per core; weight/bias replicated.
  - The host stores each 56-wide row padded to 59 with zeros.  The
    per-channel horizontal shift then folds into the input DMA for free:
    for a fixed shift dx, the shifted plane is just the flat padded plane
    read at offset +dx -- positions that fall outside [0, W) pick up the
    row padding, which is exactly the required zero padding.  The matmul
    reads [8 rows x 56 cols] strided views, never touching pad columns.
  - Channels are processed in a permuted order (grouped by c mod 7 ==
    constant shift) so each shift group is a partition-contiguous,
    channel-stride-7 affine DMA segment.  The weight matrix is permuted
    identically on the host (pure layout transform, no arithmetic), which
    leaves the GEMM result unchanged.
  - sign() runs on the Scalar engine (fp32 -> bf16; +-1 is exact in bf16
    and the 384-term accumulation is exact in fp32 PSUM, so the result is
    bit-identical to an fp32 computation).
  - GEMM: out[o, p] = sum_c wbT[c, o] * xb[c, p] on the Tensor engine,
    K = 384 contracted in 3 chunks of 128, N tiles of 448 (8 H-rows).
  - Bias add fused into the PSUM -> SBUF copy on the Vector engine.
"""

import numpy as np

import concourse.bass as bass
import concourse.tile as tile
from concourse import bacc, mybir
from concourse.bass_utils import run_bass_kernel_spmd

# Problem constants (hardcoded per spec)
B, C, H, W = 32, 384, 56, 56
PLANE = H * W              # 3136 (unpadded output plane)
NCORES = 8
BL = B // NCORES           # 4 batches per core
KS = 7                     # cyclic shift period (kernel_size 7)
NK = C // 128              # 3 contraction chunks
NM = C // 128              # 3 output-channel chunks
ROWS_PER_TILE = 8
NTILE = ROWS_PER_TILE * W  # 448 pixels per PSUM tile
NN = H // ROWS_PER_TILE    # 7 pixel tiles per (b, m)
WPAD = 59                  # row pitch: 56 data + 3 zero cols (>= max |dx|)
PLANE_P = H * WPAD         # 3304 (padded input plane)
BACK_PAD = 7 * PLANE_P     # slack so segment APs can over-claim past the end
NX_ELEMS = BL * C * PLANE_P + BACK_PAD
NOUT_ELEMS = BL * C * PLANE

# Shift-group segments of the permuted channel order.  perm = channels
# grouped by r = c mod 7 (r ascending, then c ascending within the group).
# Each segment is a partition-contiguous run inside one 128-channel chunk:
# (chunk, part_start, nseg, c_first, dx) with original channels
# c_first + 7*i for i in [0, nseg).
SEGMENTS = [
    (0, 0, 55, 0, 0),
    (0, 55, 55, 1, 1),
    (0, 110, 18, 2, 2),
    (1, 0, 37, 128, 2),    # r=2 continued: 2 + 7*18
    (1, 37, 55, 3, 3),
    (1, 92, 36, 4, -3),
    (2, 0, 19, 256, -3),   # r=4 continued: 4 + 7*36
    (2, 19, 55, 5, -2),
    (2, 74, 54, 6, -1),
]

PERM = np.concatenate([np.arange(r, C, KS) for r in range(KS)])

_COMPILED = None


def _build_program():
    """Trace + compile the single-core Bass program (same on all 8 cores)."""
    nc = bacc.Bacc(
        "TRN2",
        target_bir_lowering=False,
        debug=False,
        num_devices=NCORES,
    )
    x_d = nc.dram_tensor("x", [NX_ELEMS], mybir.dt.float32, kind="ExternalInput")
    w_d = nc.dram_tensor("wt", [C, C], mybir.dt.float32, kind="ExternalInput")
    b_d = nc.dram_tensor("bias", [C], mybir.dt.float32, kind="ExternalInput")
    o_d = nc.dram_tensor("out", [NOUT_ELEMS], mybir.dt.float32, kind="ExternalOutput")

    x_ap = x_d.ap()
    o_ap = o_d.ap()

    segs_by_chunk = [[s[1:] for s in SEGMENTS if s[0] == k] for k in range(NK)]

    with tile.TileContext(nc) as tc:
        with (
            tc.tile_pool(name="const", bufs=1) as cpool,
            tc.tile_pool(name="xbr", bufs=6) as xbr_pool,
            tc.tile_pool(name="xbc", bufs=6) as xbc_pool,
            tc.tile_pool(name="psum", bufs=8, space="PSUM") as psum_pool,
            tc.tile_pool(name="outs", bufs=3) as out_pool,
        ):
            # Binarized, pre-transposed, channel-permuted weights: wbT[c, o].
            w_bf = []
            for k in range(NK):
                wraw = cpool.tile([128, C], mybir.dt.float32, tag=f"wraw{k}")
                nc.sync.dma_start(wraw[:], w_d.ap()[128 * k : 128 * (k + 1), :])
                wb = cpool.tile([128, C], mybir.dt.bfloat16, tag=f"wb{k}")
                nc.scalar.sign(wb[:], wraw[:])
                w_bf.append(wb)

            bias_t = []
            for m in range(NM):
                bt = cpool.tile([128, 1], mybir.dt.float32, tag=f"bias{m}")
                nc.sync.dma_start(bt[:], b_d.ap()[128 * m : 128 * (m + 1)].unsqueeze(1))
                bias_t.append(bt)

            for b in range(BL):
                xbcs = []
                for k in range(NK):
                    # SWDGE load with inline fp32->bf16 cast (sign-preserving).
                    xbr = xbr_pool.tile([128, PLANE_P], mybir.dt.bfloat16, tag="xbr")
                    for (part_start, nseg, c_first, dx) in segs_by_chunk[k]:
                        base = (b * C + c_first) * PLANE_P + dx
                        src = (
                            x_ap[base : base + nseg * KS * PLANE_P]
                            .rearrange("(p q) -> p q", q=KS * PLANE_P)[:, :PLANE_P]
                        )
                        nc.gpsimd.dma_start(xbr[part_start : part_start + nseg, :], src)

                    # Binarize + drop the pad columns: strided read of the
                    # [H, :W] view, contiguous [128, H*W] write.
                    xbc = xbc_pool.tile([128, PLANE], mybir.dt.bfloat16, tag="xbc")
                    nc.scalar.sign(
                        xbc[:].rearrange("p (h w) -> p h w", w=W),
                        xbr[:].rearrange("p (h w) -> p h w", w=WPAD)[:, :, :W],
                    )
                    xbcs.append(xbc)

                for m in range(NM):
                    pss = [
                        psum_pool.tile(
                            [128, NTILE], mybir.dt.float32, tag="ps", name=f"ps{b}_{m}_{n}"
                        )
                        for n in range(NN)
                    ]
                    # k-outer: the stationary weight chunk is reused across
                    # the 7 pixel tiles; PSUM accumulates across k.
                    for k in range(NK):
                        for n in range(NN):
                            nc.tensor.matmul(
                                pss[n][:],
                                w_bf[k][:, 128 * m : 128 * (m + 1)],
                                xbcs[k][:, NTILE * n : NTILE * (n + 1)],
                                start=(k == 0),
                                stop=(k == NK - 1),
                            )
                    # Bias-add drains PSUM into one full-plane tile so the
                    # store has 12.5 KB contiguous runs per partition.
                    ot = out_pool.tile([128, PLANE], mybir.dt.float32, tag="ot")
                    for n in range(NN):
                        nc.vector.tensor_scalar_add(
                            ot[:, NTILE * n : NTILE * (n + 1)], pss[n][:], bias_t[m][:]
                        )
                    obase = (b * C + 128 * m) * PLANE
                    dst = o_ap[obase : obase + 128 * PLANE].rearrange(
                        "(p q) -> p q", q=PLANE
                    )
                    nc.gpsimd.dma_start(dst, ot[:])

    nc.compile()
    return nc


def _get_program():
    global _COMPILED
    if _COMPILED is None:
        _COMPILED = _build_program()
    return _COMPILED


# Set by test harness to request an NTFF-profiled run; results stashed here.
TRACE = False
LAST_EXEC_TIME_NS = None


def kernel(x, weight, bias):
    global LAST_EXEC_TIME_NS
    x = np.ascontiguousarray(np.asarray(x, dtype=np.float32))
    weight = np.asarray(weight, dtype=np.float32)
    bias = np.ascontiguousarray(np.asarray(bias, dtype=np.float32))

    # Pure layout transform (no arithmetic): transpose + channel-permute the
    # weight so device partition p of contraction chunk k holds original
    # channel PERM[128k + p], matching the activation segment layout.
    wtp = np.ascontiguousarray(weight[:, PERM].T)

    nc = _get_program()

    in_maps = []
    for i in range(NCORES):
        xi = np.zeros(NX_ELEMS, dtype=np.float32)
        view = xi[: BL * C * PLANE_P].reshape(BL, C, H, WPAD)
        view[..., :W] = x[i * BL : (i + 1) * BL]
        in_maps.append({"x": xi, "wt": wtp, "bias": bias})

    res = run_bass_kernel_spmd(
        nc, in_maps, list(range(NCORES)), trace=TRACE
    )
    LAST_EXEC_TIME_NS = res.exec_time_ns

    out = np.empty((B, C, H, W), dtype=np.float32)
    for i in range(NCORES):
        out[i * BL : (i + 1) * BL] = res.results[i]["out"].reshape(BL, C, H, W)
    return out


# revision 12
# speedup vs baseline: 1.8479x; 1.1447x over previous
"""CycleFC (1-bit weights/activations) Trainium2 kernel.

Computes, for x (B=32, C=384, H=56, W=56), weight (C, C), bias (C,):
    xb = sign(x); wb = sign(weight)
    shifted[b,c,h,w] = xb[b,c,h,w+dx_c]  (0 outside [0,W)), dx_c = (c+3)%7-3
    out = einsum('bchw,oc->bohw', shifted, wb) + bias

Strategy (8 NeuronCores, SPMD):
  - Data-parallel over batch: 4 batches per core; weight/bias replicated.
  - The host stores each 56-wide row padded to 59 with zeros.  The
    per-channel horizontal shift then folds into the input DMA for free:
    for a fixed shift dx, the shifted plane is just the flat padded plane
    read at offset +dx -- positions that fall outside [0, W) pick up the
    row padding, which is exactly the required zero padding.  The matmul
    reads [8 rows x 56 cols] strided views, never touching pad columns.
  - Channels are processed in a permuted order (grouped by c mod 7 ==
    constant shift) so each shift group is a partition-contiguous,
    channel-stride-7 affine DMA segment.  The weight matrix is permuted
    identically on the host (pure layout transform, no arithmetic), which
    leaves the GEMM result unchanged.
  - sign() runs on the Scalar engine (fp32 -> bf16; +-1 is exact in bf16
    and the 384-term accumulation is exact in fp32 PSUM, so the result is
    bit-identical to an fp32 computation).
  - GEMM: out[o, p] = sum_c wbT[c, o] * xb[c, p] on the Tensor engine,
    K = 384 contracted in 3 chunks of 128, N tiles of 448 (8 H-rows).
  - Bias add fused into the PSUM -> SBUF copy on the Vector engine.
"""

import numpy as np

import concourse.bass as bass
import concourse.tile as tile
from concourse import bacc, mybir
from concourse.bass_utils import run_bass_kernel_spmd

# Problem constants (hardcoded per spec)
B, C, H, W = 32, 384, 56, 56
PLANE = H * W              # 3136 (unpadded output plane)
NCORES = 8
BL = B // NCORES           # 4 batches per core
KS = 7                     # cyclic shift period (kernel_size 7)
NK = C // 128              # 3 contraction chunks
NM = C // 128              # 3 output-channel chunks
ROWS_PER_TILE = 8
NTILE = ROWS_PER_TILE * W  # 448 pixels per PSUM tile
NN = H // ROWS_PER_TILE    # 7 pixel tiles per (b, m)
WPAD = 59                  # row pitch: 56 data + 3 zero cols (>= max |dx|)
PLANE_P = H * WPAD         # 3304 (padded input plane)
BACK_PAD = 7 * PLANE_P     # slack so segment APs can over-claim past the end
NX_ELEMS = BL * C * PLANE_P + BACK_PAD
NOUT_ELEMS = BL * C * PLANE

# Shift-group segments of the permuted channel order.  perm = channels
# grouped by r = c mod 7 (r ascending, then c ascending within the group).
# Each segment is a partition-contiguous run inside one 128-channel chunk:
# (chunk, part_start, nseg, c_first, dx) with original channels
# c_first + 7*i for i in [0, nseg).
SEGMENTS = [
    (0, 0, 55, 0, 0),
    (0, 55, 55, 1, 1),
    (0, 110, 18, 2, 2),
    (1, 0, 37, 128, 2),    # r=2 continued: 2 + 7*18
    (1, 37, 55, 3, 3),
    (1, 92, 36, 4, -3),
    (2, 0, 19, 256, -3),   # r=4 continued: 4 + 7*36
    (2, 19, 55, 5, -2),
    (2, 74, 54, 6, -1),
]

PERM = np.concatenate([np.arange(r, C, KS) for r in range(KS)])

_COMPILED = None


def _build_program():
    """Trace + compile the single-core Bass program (same on all 8 cores)."""
    nc = bacc.Bacc(
        "TRN2",
        target_bir_lowering=False,
        debug=False,
        num_devices=NCORES,
    )
    x_d = nc.dram_tensor("x", [NX_ELEMS], mybir.dt.float32, kind="ExternalInput")
    w_d = nc.dram_tensor("wt", [C, C], mybir.dt.float32, kind="ExternalInput")
    b_d = nc.dram_tensor("bias", [C], mybir.dt.float32, kind="ExternalInput")
    o_d = nc.dram_tensor("out", [NOUT_ELEMS], mybir.dt.float32, kind="ExternalOutput")

    x_ap = x_d.ap()
    o_ap = o_d.ap()

    segs_by_chunk = [[s[1:] for s in SEGMENTS if s[0] == k] for k in range(NK)]

    with tile.TileContext(nc) as tc:
        with (
            tc.tile_pool(name="const", bufs=1) as cpool,
            tc.tile_pool(name="xbr", bufs=6) as xbr_pool,
            tc.tile_pool(name="xbc", bufs=6) as xbc_pool,
            tc.tile_pool(name="psum", bufs=8, space="PSUM") as psum_pool,
            tc.tile_pool(name="outs", bufs=4) as out_pool,
        ):
            # Binarized, pre-transposed, channel-permuted weights: wbT[c, o].
            w_bf = []
            for k in range(NK):
                wraw = cpool.tile([128, C], mybir.dt.float32, tag=f"wraw{k}")
                nc.sync.dma_start(wraw[:], w_d.ap()[128 * k : 128 * (k + 1), :])
                wb = cpool.tile([128, C], mybir.dt.bfloat16, tag=f"wb{k}")
                nc.scalar.sign(wb[:], wraw[:])
                w_bf.append(wb)

            bias_t = []
            for m in range(NM):
                bt = cpool.tile([128, 1], mybir.dt.float32, tag=f"bias{m}")
                nc.sync.dma_start(bt[:], b_d.ap()[128 * m : 128 * (m + 1)].unsqueeze(1))
                bias_t.append(bt)

            xbrs = {}

            def emit_loads(b):
                # SWDGE loads with inline fp32->bf16 cast (sign-preserving).
                tiles = []
                for k in range(NK):
                    xbr = xbr_pool.tile(
                        [128, PLANE_P], mybir.dt.bfloat16, tag="xbr", name=f"xbr{b}_{k}"
                    )
                    for (part_start, nseg, c_first, dx) in segs_by_chunk[k]:
                        base = (b * C + c_first) * PLANE_P + dx
                        src = (
                            x_ap[base : base + nseg * KS * PLANE_P]
                            .rearrange("(p q) -> p q", q=KS * PLANE_P)[:, :PLANE_P]
                        )
                        nc.gpsimd.dma_start(xbr[part_start : part_start + nseg, :], src)
                    tiles.append(xbr)
                xbrs[b] = tiles

            # Software pipeline: keep 2 batches of loads in flight so the
            # Scalar/Tensor engines never starve between batch iterations.
            emit_loads(0)
            emit_loads(1)

            for b in range(BL):
                xbcs = []
                for k in range(NK):
                    # Binarize + drop the pad columns: strided read of the
                    # [H, :W] view, contiguous [128, H*W] write.
                    xbc = xbc_pool.tile(
                        [128, PLANE], mybir.dt.bfloat16, tag="xbc", name=f"xbc{b}_{k}"
                    )
                    nc.scalar.sign(
                        xbc[:].rearrange("p (h w) -> p h w", w=W),
                        xbrs[b][k][:].rearrange("p (h w) -> p h w", w=WPAD)[:, :, :W],
                    )
                    xbcs.append(xbc)
                del xbrs[b]

                for m in range(NM):
                    pss = [
                        psum_pool.tile(
                            [128, NTILE], mybir.dt.float32, tag="ps", name=f"ps{b}_{m}_{n}"
                        )
                        for n in range(NN)
                    ]
                    # k-outer: the stationary weight chunk is reused across
                    # the 7 pixel tiles; PSUM accumulates across k.
                    for k in range(NK):
                        for n in range(NN):
                            nc.tensor.matmul(
                                pss[n][:],
                                w_bf[k][:, 128 * m : 128 * (m + 1)],
                                xbcs[k][:, NTILE * n : NTILE * (n + 1)],
                                start=(k == 0),
                                stop=(k == NK - 1),
                            )
                    # Bias-add drains PSUM into one full-plane tile so the
                    # store has 12.5 KB contiguous runs per partition.
                    ot = out_pool.tile(
                        [128, PLANE], mybir.dt.float32, tag="ot", name=f"ot{b}_{m}"
                    )
                    for n in range(NN):
                        nc.vector.tensor_scalar_add(
                            ot[:, NTILE * n : NTILE * (n + 1)], pss[n][:], bias_t[m][:]
                        )
                    obase = (b * C + 128 * m) * PLANE
                    dst = o_ap[obase : obase + 128 * PLANE].rearrange(
                        "(p q) -> p q", q=PLANE
                    )
                    # Stores ride the Sync engine's HWDGE ring: store traffic
                    # never head-of-line-blocks the SWDGE load rings.
                    nc.sync.dma_start(dst, ot[:])

                if b + 2 < BL:
                    emit_loads(b + 2)

    nc.compile()
    return nc


def _get_program():
    global _COMPILED
    if _COMPILED is None:
        _COMPILED = _build_program()
    return _COMPILED


# Set by test harness to request an NTFF-profiled run; results stashed here.
TRACE = False
LAST_EXEC_TIME_NS = None


def kernel(x, weight, bias):
    global LAST_EXEC_TIME_NS
    x = np.ascontiguousarray(np.asarray(x, dtype=np.float32))
    weight = np.asarray(weight, dtype=np.float32)
    bias = np.ascontiguousarray(np.asarray(bias, dtype=np.float32))

    # Pure layout transform (no arithmetic): transpose + channel-permute the
    # weight so device partition p of contraction chunk k holds original
    # channel PERM[128k + p], matching the activation segment layout.
    wtp = np.ascontiguousarray(weight[:, PERM].T)

    nc = _get_program()

    in_maps = []
    for i in range(NCORES):
        xi = np.zeros(NX_ELEMS, dtype=np.float32)
        view = xi[: BL * C * PLANE_P].reshape(BL, C, H, WPAD)
        view[..., :W] = x[i * BL : (i + 1) * BL]
        in_maps.append({"x": xi, "wt": wtp, "bias": bias})

    res = run_bass_kernel_spmd(
        nc, in_maps, list(range(NCORES)), trace=TRACE
    )
    LAST_EXEC_TIME_NS = res.exec_time_ns

    out = np.empty((B, C, H, W), dtype=np.float32)
    for i in range(NCORES):
        out[i * BL : (i + 1) * BL] = res.results[i]["out"].reshape(BL, C, H, W)
    return out
